# revision 9
# baseline (speedup 1.0000x reference)
"""Cost-sensitive loss (CE + cost-matrix lookup) on Trainium2, 8-core data-parallel.

Device work (per core, shard of 32768 rows x 1000 classes, fp32):
  - Stream x in batches of 8 [128, 1000] tiles: one 4 MB HWDGE DMA per
    batch, rows permuted so each partition's line is 32 KB contiguous in
    HBM. Rings alternate sync/scalar so transfers pipeline.
  - ACT: exp(x) per tile with fp32 accum_out -> per-row sum(exp)
    (|x| <= ~6 so no max-shift needed); the activation OUTPUT (normally
    wasted) is written as fp16 -> esc, an order-preserving all-positive
    argmax key (the DVE runs every op at ~1 elem/cycle/lane, so the
    argmax budget is a single full pass over the data).
  - DVE (the one full pass): grouped reduce_max over 25 blocks of 40
    -> gm [128, 25] fp16 per tile. Then two small batched ops per
    8-tile batch: z = gm_bits * 32 + block_id (scalar_tensor_tensor on
    the uint16 view; positive fp16 bit patterns sort like the values),
    and a grouped reduce_max z [128, 8, 25] -> zb [128, 8] uint32.
    max(z) is lexicographic (block max, block id), so zb & 31 is the
    argmax block exactly (fp16 ties resolve to the largest block id).
  - Epilogue: ls = ln(s_acc); partials[128,1] = sum_t ls; DMA out
    partials + zb table.

Host work (O(N) with small constants):
  - decode winning block, exact fp32 argmax within the 40-wide block,
    x[row, label[row]] extraction, cost_matrix[label, pred] lookup,
    final sums / division by N.

fp16 rounding only affects which near-tied BLOCK wins (~0.4% of rows);
within the block the host argmax is exact. The cost-term perturbation
is ~1e-4 absolute, three orders below the 2e-2 relative tolerance.
"""

import numpy as np

import concourse.bacc as bacc
import concourse.bass as bass  # noqa: F401  (kept for API parity)
import concourse.mybir as mybir
import concourse.tile as tile
from concourse import bass_utils

N = 262144
C = 1000
NCORES = 8
NS = N // NCORES          # 32768 rows per core
P = 128
TPB = 8                   # tiles per DMA batch
NT = NS // P              # 256 tiles per core
NB = NT // TPB            # 32 batches per core
G = 25                    # blocks per row
W = C // G                # block width (40)
S = 32                    # block-id stride in the packed key

F32 = mybir.dt.float32
F16 = mybir.dt.float16
U16 = mybir.dt.uint16
U32 = mybir.dt.uint32

_CACHE: dict = {}


def _body(tc, nc, x, blkc, partials, zb_out):
    from contextlib import ExitStack

    AX = mybir.AxisListType.X
    ALU = mybir.AluOpType
    EXP = mybir.ActivationFunctionType.Exp
    LN = mybir.ActivationFunctionType.Ln

    # Row layout: row = b*1024 + p*8 + j  (batch, partition, tile-in-batch)
    # -> per partition each batch is 8 consecutive HBM rows = 32 KB contiguous.
    x_b = x.ap().rearrange("(b p j) c -> p b (j c)", b=NB, p=P, j=TPB)

    with ExitStack() as ctx:
        const = ctx.enter_context(tc.tile_pool(name="const", bufs=1))
        s_acc = const.tile([P, NT], F32)
        zb_acc = const.tile([P, NT], U32)
        blkc_sb = const.tile([P, TPB * G], U32)
        nc.sync.dma_start(out=blkc_sb[:], in_=blkc.ap())

        xp = ctx.enter_context(tc.tile_pool(name="xp", bufs=4))
        ep = ctx.enter_context(tc.tile_pool(name="ep", bufs=3))
        wk = ctx.enter_context(tc.tile_pool(name="wk", bufs=3))

        for b in range(NB):
            xt = xp.tile([P, TPB * C], F32, tag="xt")
            # Split each batch across both HWDGE rings so neither idles.
            half = TPB // 2 * C
            x_bv = x_b[:, b, :].rearrange("p (h jc) -> p h jc", h=2)
            nc.sync.dma_start(out=xt[:, :half], in_=x_bv[:, 0, :])
            nc.gpsimd.dma_start(out=xt[:, half:], in_=x_bv[:, 1, :])

            esc = ep.tile([P, TPB * C], F16, tag="esc")
            gm = wk.tile([P, TPB * G], F16, tag="gm")
            z = wk.tile([P, TPB * G], U32, tag="z")
            for j in range(TPB):
                t = b * TPB + j
                nc.scalar.activation(
                    out=esc[:, j * C:(j + 1) * C],
                    in_=xt[:, j * C:(j + 1) * C],
                    func=EXP,
                    accum_out=s_acc[:, t:t + 1],
                )
                nc.vector.reduce_max(
                    out=gm[:, j * G:(j + 1) * G],
                    in_=esc[:, j * C:(j + 1) * C].rearrange(
                        "p (g w) -> p g w", w=W
                    ),
                    axis=AX,
                )
            # z = gm_bits*S + blk ; max(z) = (block max, block id) lex.
            nc.vector.scalar_tensor_tensor(
                out=z[:],
                in0=gm[:].bitcast(U16),
                scalar=S,
                in1=blkc_sb[:],
                op0=ALU.mult,
                op1=ALU.add,
            )
            nc.vector.reduce_max(
                out=zb_acc[:, b * TPB:(b + 1) * TPB],
                in_=z[:].rearrange("p (j g) -> p j g", g=G),
                axis=AX,
            )

        # Epilogue: per-partition sum of log(sumexp).
        ls = const.tile([P, NT], F32)
        nc.scalar.activation(out=ls[:], in_=s_acc[:], func=LN)
        p1 = const.tile([P, 1], F32)
        nc.vector.reduce_sum(out=p1[:], in_=ls[:], axis=AX)
        nc.sync.dma_start(out=partials.ap(), in_=p1[:])
        nc.sync.dma_start(out=zb_out.ap(), in_=zb_acc[:])


def build_module():
    nc = bacc.Bacc(
        "TRN2",
        target_bir_lowering=False,
        debug=False,
        enable_asserts=False,
        num_devices=NCORES,
    )
    x = nc.dram_tensor("x", [NS, C], F32, kind="ExternalInput")
    blkc = nc.dram_tensor("blkc", [P, TPB * G], U32, kind="ExternalInput")
    partials = nc.dram_tensor("partials", [P, 1], F32, kind="ExternalOutput")
    zb_out = nc.dram_tensor("zb_out", [P, NT], U32, kind="ExternalOutput")
    with tile.TileContext(nc) as tc:
        _body(tc, nc, x, blkc, partials, zb_out)
    nc.compile()
    return nc


def host_inputs(ncores=NCORES, x=None):
    """Per-core input maps. x is the full [N, C] fp32 array."""
    blkc = np.broadcast_to(
        np.tile(np.arange(G, dtype=np.uint32), TPB), (P, TPB * G)
    ).copy()
    return [
        {"x": x[cidx * NS:(cidx + 1) * NS], "blkc": blkc}
        for cidx in range(ncores)
    ]


def combine(results, x, lab, cost_matrix):
    """Host-side finish: ce = sum(log sumexp) - sum(x[label]); cost lookup."""
    n_total = len(results) * NS
    lse_sum = 0.0
    blk_all = []
    for r in results:
        lse_sum += np.asarray(r["partials"], dtype=np.float64).sum()
        zb = np.asarray(r["zb_out"]).astype(np.int64)         # [P, NT]
        blk = zb & (S - 1)                                    # winning block id
        # col t = b*8 + j; row = b*1024 + p*8 + j
        wv = blk.reshape(P, NB, TPB)
        blk_all.append(np.transpose(wv, (1, 0, 2)).reshape(-1))
    blk = np.clip(np.concatenate(blk_all), 0, G - 1)          # [N]
    # Exact fp32 argmax within the winning 40-wide block.
    base = blk * W
    rows = np.arange(n_total, dtype=np.int64)[:, None]
    inner = np.argmax(x[rows, base[:, None] + np.arange(W)[None, :]], axis=1)
    preds = base + inner
    xlab_sum = np.take_along_axis(
        x, lab[:, None].astype(np.int64), axis=1
    )[:, 0].astype(np.float64).sum()
    cost_sum = np.asarray(cost_matrix)[
        lab.astype(np.int64), preds
    ].astype(np.float64).sum()
    ce = (lse_sum - xlab_sum) / n_total
    cost = cost_sum / n_total
    return np.float32(ce + cost)


def kernel(outputs, labels, cost_matrix):
    if "nc" not in _CACHE:
        _CACHE["nc"] = build_module()
    nc = _CACHE["nc"]
    x = np.ascontiguousarray(np.asarray(outputs), dtype=np.float32)
    lab = np.asarray(labels)
    in_maps = host_inputs(x=x)
    res = bass_utils.run_bass_kernel_spmd(nc, in_maps, core_ids=list(range(NCORES)))
    return combine(res.results, x, lab, cost_matrix)


# revision 10
# speedup vs baseline: 1.3226x; 1.3226x over previous
"""Cost-sensitive loss (CE + cost-matrix lookup) on Trainium2, 8-core data-parallel.

Device work (per core, shard of 32768 rows x 1000 classes, fp32):
  - Stream x in batches of 8 [128, 1000] tiles: one 4 MB HWDGE DMA per
    batch, rows permuted so each partition's line is 32 KB contiguous in
    HBM. Rings alternate sync/scalar so transfers pipeline.
  - ACT: exp(x) per tile with fp32 accum_out -> per-row sum(exp)
    (|x| <= ~6 so no max-shift needed); the activation OUTPUT (normally
    wasted) is written as fp16 -> esc, an order-preserving all-positive
    argmax key (the DVE runs every op at ~1 elem/cycle/lane, so the
    argmax budget is a single full pass over the data).
  - DVE (the one full pass): grouped reduce_max over 25 blocks of 40
    -> gm [128, 25] fp16 per tile. Then two small batched ops per
    8-tile batch: z = gm_bits * 32 + block_id (scalar_tensor_tensor on
    the uint16 view; positive fp16 bit patterns sort like the values),
    and a grouped reduce_max z [128, 8, 25] -> zb [128, 8] uint32.
    max(z) is lexicographic (block max, block id), so zb & 31 is the
    argmax block exactly (fp16 ties resolve to the largest block id).
  - Epilogue: ls = ln(s_acc); partials[128,1] = sum_t ls; DMA out
    partials + zb table.

Host work (O(N) with small constants):
  - decode winning block, exact fp32 argmax within the 40-wide block,
    x[row, label[row]] extraction, cost_matrix[label, pred] lookup,
    final sums / division by N.

fp16 rounding only affects which near-tied BLOCK wins (~0.4% of rows);
within the block the host argmax is exact. The cost-term perturbation
is ~1e-4 absolute, three orders below the 2e-2 relative tolerance.
"""

import numpy as np

import concourse.bacc as bacc
import concourse.bass as bass  # noqa: F401  (kept for API parity)
import concourse.mybir as mybir
import concourse.tile as tile
from concourse import bass_utils

N = 262144
C = 1000
NCORES = 8
NS = N // NCORES          # 32768 rows per core
P = 128
TPB = 8                   # tiles per DMA batch
NT = NS // P              # 256 tiles per core
NB = NT // TPB            # 32 batches per core
G = 25                    # blocks per row
W = C // G                # block width (40)
S = 32                    # block-id stride in the packed key

F32 = mybir.dt.float32
F16 = mybir.dt.float16
U16 = mybir.dt.uint16
U32 = mybir.dt.uint32

_CACHE: dict = {}


def _body(tc, nc, x, blkc, partials, zb_out):
    from contextlib import ExitStack

    AX = mybir.AxisListType.X
    ALU = mybir.AluOpType
    EXP = mybir.ActivationFunctionType.Exp
    LN = mybir.ActivationFunctionType.Ln

    # Row layout: row = b*1024 + p*8 + j  (batch, partition, tile-in-batch)
    # -> per partition each batch is 8 consecutive HBM rows = 32 KB contiguous.
    x_b = x.ap().rearrange("(b p j) c -> p b (j c)", b=NB, p=P, j=TPB)

    with ExitStack() as ctx:
        const = ctx.enter_context(tc.tile_pool(name="const", bufs=1))
        s_acc = const.tile([P, NT], F32)
        zb_acc = const.tile([P, NT], U32)
        blkc_sb = const.tile([P, TPB * G], U32)
        nc.sync.dma_start(out=blkc_sb[:], in_=blkc.ap())

        xp = ctx.enter_context(tc.tile_pool(name="xp", bufs=3))
        ep = ctx.enter_context(tc.tile_pool(name="ep", bufs=3))
        wk = ctx.enter_context(tc.tile_pool(name="wk", bufs=3))

        for b in range(NB):
            xt = xp.tile([P, TPB * C], F32, tag="xt")
            # Split each batch across both HWDGE rings so neither idles.
            half = TPB // 2 * C
            x_bv = x_b[:, b, :].rearrange("p (h jc) -> p h jc", h=2)
            nc.sync.dma_start(out=xt[:, :half], in_=x_bv[:, 0, :])
            nc.scalar.dma_start(out=xt[:, half:], in_=x_bv[:, 1, :])

            esc = ep.tile([P, TPB * C], F16, tag="esc")
            gm = wk.tile([P, TPB * G], F16, tag="gm")
            z = wk.tile([P, TPB * G], U32, tag="z")
            for j in range(TPB):
                t = b * TPB + j
                nc.scalar.activation(
                    out=esc[:, j * C:(j + 1) * C],
                    in_=xt[:, j * C:(j + 1) * C],
                    func=EXP,
                    accum_out=s_acc[:, t:t + 1],
                )
                nc.vector.reduce_max(
                    out=gm[:, j * G:(j + 1) * G],
                    in_=esc[:, j * C:(j + 1) * C].rearrange(
                        "p (g w) -> p g w", w=W
                    ),
                    axis=AX,
                )
            # z = gm_bits*S + blk ; max(z) = (block max, block id) lex.
            nc.vector.scalar_tensor_tensor(
                out=z[:],
                in0=gm[:].bitcast(U16),
                scalar=S,
                in1=blkc_sb[:],
                op0=ALU.mult,
                op1=ALU.add,
            )
            nc.vector.reduce_max(
                out=zb_acc[:, b * TPB:(b + 1) * TPB],
                in_=z[:].rearrange("p (j g) -> p j g", g=G),
                axis=AX,
            )

        # Epilogue: per-partition sum of log(sumexp).
        ls = const.tile([P, NT], F32)
        nc.scalar.activation(out=ls[:], in_=s_acc[:], func=LN)
        p1 = const.tile([P, 1], F32)
        nc.vector.reduce_sum(out=p1[:], in_=ls[:], axis=AX)
        nc.sync.dma_start(out=partials.ap(), in_=p1[:])
        nc.sync.dma_start(out=zb_out.ap(), in_=zb_acc[:])


def build_module():
    nc = bacc.Bacc(
        "TRN2",
        target_bir_lowering=False,
        debug=False,
        enable_asserts=False,
        num_devices=NCORES,
    )
    x = nc.dram_tensor("x", [NS, C], F32, kind="ExternalInput")
    blkc = nc.dram_tensor("blkc", [P, TPB * G], U32, kind="ExternalInput")
    partials = nc.dram_tensor("partials", [P, 1], F32, kind="ExternalOutput")
    zb_out = nc.dram_tensor("zb_out", [P, NT], U32, kind="ExternalOutput")
    with tile.TileContext(nc) as tc:
        _body(tc, nc, x, blkc, partials, zb_out)
    nc.compile()
    return nc


def host_inputs(ncores=NCORES, x=None):
    """Per-core input maps. x is the full [N, C] fp32 array."""
    blkc = np.broadcast_to(
        np.tile(np.arange(G, dtype=np.uint32), TPB), (P, TPB * G)
    ).copy()
    return [
        {"x": x[cidx * NS:(cidx + 1) * NS], "blkc": blkc}
        for cidx in range(ncores)
    ]


def combine(results, x, lab, cost_matrix):
    """Host-side finish: ce = sum(log sumexp) - sum(x[label]); cost lookup."""
    n_total = len(results) * NS
    lse_sum = 0.0
    blk_all = []
    for r in results:
        lse_sum += np.asarray(r["partials"], dtype=np.float64).sum()
        zb = np.asarray(r["zb_out"]).astype(np.int64)         # [P, NT]
        blk = zb & (S - 1)                                    # winning block id
        # col t = b*8 + j; row = b*1024 + p*8 + j
        wv = blk.reshape(P, NB, TPB)
        blk_all.append(np.transpose(wv, (1, 0, 2)).reshape(-1))
    blk = np.clip(np.concatenate(blk_all), 0, G - 1)          # [N]
    # Exact fp32 argmax within the winning 40-wide block.
    base = blk * W
    rows = np.arange(n_total, dtype=np.int64)[:, None]
    inner = np.argmax(x[rows, base[:, None] + np.arange(W)[None, :]], axis=1)
    preds = base + inner
    xlab_sum = np.take_along_axis(
        x, lab[:, None].astype(np.int64), axis=1
    )[:, 0].astype(np.float64).sum()
    cost_sum = np.asarray(cost_matrix)[
        lab.astype(np.int64), preds
    ].astype(np.float64).sum()
    ce = (lse_sum - xlab_sum) / n_total
    cost = cost_sum / n_total
    return np.float32(ce + cost)


def kernel(outputs, labels, cost_matrix):
    if "nc" not in _CACHE:
        _CACHE["nc"] = build_module()
    nc = _CACHE["nc"]
    x = np.ascontiguousarray(np.asarray(outputs), dtype=np.float32)
    lab = np.asarray(labels)
    in_maps = host_inputs(x=x)
    res = bass_utils.run_bass_kernel_spmd(nc, in_maps, core_ids=list(range(NCORES)))
    return combine(res.results, x, lab, cost_matrix)
